# revision 47
# baseline (speedup 1.0000x reference)
"""Trainium2 Bass kernel for a causal AttentionBlock (dense transformer).

Model (reference):
    qkv = x @ Wqkv + bqkv ; 16-head causal attention (no out-proj)
    x2  = x + attn_out
    out = x2 + relu(x2 @ W1 + b1) @ W2 + b2

x: [2, 2048, 1024] fp32. 8 NeuronCores.

Sharding (no collectives): data-parallel over (batch, query-chunk). Core c
handles batch b = c//4 and the balanced causal chunk pair (j, 7-j), j = c%4,
of 8x256-row chunks, giving every core the same 512 query rows. Each core
redundantly projects K/V for its whole batch (uniform SPMD program), computes
attention for its rows with shipped additive gates/mask, then the MLP.

Everything on-chip runs transposed ([feature, row] layout). The host ships
x pre-transposed (fp32 for the residual path, fp16 for matmul operands) and
transposes the output back, so the PE does no transposes at all. All matmul
operands are fp16 (same 1 cycle/row as fp32r, same ~11-bit mantissa, half
the bytes). Weights are host-packed so every DMA is contiguous full rows.

Attention flow per head: scoresT = kT.T @ qT (PSUM fp32), Exp on ACT with
per-block additive gate bias (diag blocks get a DVE mask-add first), AV
accumulates [v|1]-augmented values so softmax denominators come free. Odd
heads use [1|v] packing and accumulate into PSUM partitions 63:128 so the
normalize chain runs at partitions 0:64 for both heads of a feature block
(PE outputs must be partition-0-aligned for >64-row results); odd-head
results shift to partitions 64:128 with a gpsimd SBUF-SBUF DMA, then the
residual add happens partition-aligned up there. 1/den is computed as
exp(-ln(den)) on ACT
(ln+exp share an ACT table with Exp; Reciprocal does not and would thrash
1.3us table loads). K-projection for step f+1 and the second half of the
V-projection are interleaved into attention(f)'s instruction stream to keep
the PE continuously busy (the HAM clock gate halves the PE clock after any
~3.4us window with idle gaps).
"""
import os
import sys

sys.path.insert(0, "/opt/trn_rl_repo")

import numpy as np

import bass_rust
import concourse.bass as bass
import concourse.mybir as mybir
import concourse.tile as tile
from concourse.bass_utils import run_bass_kernel_spmd

# ---------------------------------------------------------------- constants
B, T, N = 2, 2048, 1024
H, D = 16, 64
NCORES = 8
CH = 256               # query chunk rows
F32 = mybir.dt.float32
F32R = mybir.dt.float32r
F16 = mybir.dt.float16
F8 = mybir.dt.float8e4
U16 = mybir.dt.uint16
U32 = mybir.dt.uint32

# fp8(e4m3) DoubleRow matmuls for the MLP: 2 contraction rows per partition
# per pass. Weights are host-scaled by 64 into e4m3's normal range; the
# PSUM-evacuation activations descale by 1/64.
FP8_MLP = os.environ.get("KERNEL_FP8_MLP", "0") == "1"
W8SCALE = 64.0

_prog_cache = {}


# ------------------------------------------------------------- wait legalizer
def _legalize_waits(nc):
    """This walrus build accepts <=1 sync wait on most instructions and 0 on
    4-byte-input Matmult (fused self-loading LDW). Move excess waits onto bare
    EventSemaphore instructions inserted before, on the same engine."""
    n_split = 0
    for fn in nc.m.functions:
        for blk in fn.blocks:
            insts = blk.instructions
            out = []
            for inst in insts:
                si = inst.sync_info
                waits = list(si.on_wait) if si is not None else []
                tname = type(inst).__name__
                if tname in ("InstMatmult", "InstMatmultMx"):
                    maxw = 0
                    for arg in inst.ins:
                        dt = getattr(arg, "dtype", None)
                        if dt is not None and mybir.dt.size(dt) == 2:
                            maxw = 1
                            break
                else:
                    maxw = 1
                if len(waits) > maxw:
                    extra = waits[:-maxw] if maxw else waits
                    keep = waits[-maxw:] if maxw else []
                    for k, w in enumerate(extra):
                        ev = mybir.InstEventSemaphore(
                            name=f"{inst.name}-lw{k}", ins=[], outs=[]
                        )
                        ev.engine = inst.engine
                        ev.sync_info = bass_rust.SyncInfo(on_wait=[w], on_update=[])
                        out.append(ev)
                        n_split += 1
                    inst.sync_info = bass_rust.SyncInfo(
                        on_wait=keep, on_update=list(si.on_update)
                    )
                out.append(inst)
            insts[:] = out
    return n_split


# ------------------------------------------------------------------- program
def _build_program():
    nc = bass.Bass("TRN2", debug=False, num_devices=NCORES)

    t_ = {}
    t_["xqt"] = nc.dram_tensor("xqt", [N, 2 * CH], F32, kind="ExternalInput").ap()
    t_["xqtb"] = nc.dram_tensor("xqtb", [N, 2 * CH], F16,
                                kind="ExternalInput").ap()
    t_["xbt"] = nc.dram_tensor("xbt", [N, T], F16, kind="ExternalInput").ap()
    t_["wq_t"] = nc.dram_tensor("wq_t", [8, 128, 8, 128], F16,
                                kind="ExternalInput").ap()
    t_["wk_t"] = nc.dram_tensor("wk_t", [8, 128, 8, 128], F16,
                                kind="ExternalInput").ap()
    t_["wv_t"] = nc.dram_tensor("wv_t", [4, 8, 128, 256], F16,
                                kind="ExternalInput").ap()
    if FP8_MLP:
        t_["w1_t"] = nc.dram_tensor("w1_t", [32, 128, 4, 2, 128], F8,
                                    kind="ExternalInput").ap()
        t_["w2_t"] = nc.dram_tensor("w2_t", [8, 128, 16, 2, 128], F8,
                                    kind="ExternalInput").ap()
    else:
        t_["w1_t"] = nc.dram_tensor("w1_t", [32, 128, 8, 128], F16,
                                    kind="ExternalInput").ap()
        t_["w2_t"] = nc.dram_tensor("w2_t", [8, 128, 32, 128], F16,
                                    kind="ExternalInput").ap()
    for nm, w in (("bqs", 8), ("bk", 8), ("bv", 8), ("b1", 32), ("b2", 8)):
        t_[nm] = nc.dram_tensor(nm, [128, w], F32, kind="ExternalInput").ap()
    t_["gates"] = nc.dram_tensor("gates", [128, 16], F32,
                                 kind="ExternalInput").ap()
    t_["maskd"] = nc.dram_tensor("maskd", [256, CH], F32,
                                 kind="ExternalInput").ap()
    t_["out_t"] = nc.dram_tensor("out_t", [N, 2 * CH], F32,
                                 kind="ExternalOutput").ap()

    with tile.TileContext(nc) as tc:
        _emit(nc, tc, t_)
    return nc


def _emit(nc, tc, t_):
    AF = mybir.ActivationFunctionType
    OP = mybir.AluOpType

    with tc.tile_pool(name="const", bufs=1) as const:
        ones_r = const.tile([128, 64], F32R)
        nc.vector.memset(ones_r[:].bitcast(U32), 0x3F800000)
        bias = {}
        for nm, w in (("bqs", 8), ("bk", 8), ("bv", 8), ("b1", 32), ("b2", 8)):
            bias[nm] = const.tile([128, w], F32, name=f"b_{nm}")
            nc.sync.dma_start(bias[nm][:], t_[nm])
        gt = const.tile([128, 16], F32)
        nc.sync.dma_start(gt[:], t_["gates"])
        md = const.tile([128, 2, CH], F32)
        nc.sync.dma_start(md[:], t_["maskd"].rearrange("(c p) q -> p c q", p=128))

        with tc.tile_pool(name="outer", bufs=1) as pout:
            x2t = [pout.tile([128, 2 * CH], F32, tag=f"x2t{f}", name=f"x2t{f}")
                   for f in range(8)]
            if FP8_MLP:
                # MLP operand, DoubleRow-paired: x2m[kc2][:, i, :] holds
                # feature block 2*kc2+i
                x2m = [pout.tile([128, 2, 2 * CH], F8, tag=f"x2m{i}",
                                 name=f"x2m{i}") for i in range(4)]
                x2r = [x2m[f // 2][:, f % 2, :] for f in range(8)]
            else:
                x2r = [pout.tile([128, 2 * CH], F16, tag=f"x2r{f}",
                                 name=f"x2r{f}") for f in range(8)]

            with tc.tile_pool(name="keep", bufs=1) as keep, \
                 tc.tile_pool(name="pw", bufs=2) as pw, \
                 tc.tile_pool(name="p2w", bufs=2) as p2w, \
                 tc.tile_pool(name="psP", bufs=2, space="PSUM") as psP, \
                 tc.tile_pool(name="psS", bufs=2, space="PSUM") as psS, \
                 tc.tile_pool(name="psA", bufs=3, space="PSUM") as psA, \
                 tc.tile_pool(name="psB", bufs=1, space="PSUM") as psB:
                _phase12(nc, tc, AF, OP, t_, bias, gt, md, ones_r,
                         keep, pw, p2w, psP, psS, psA, psB, x2t, x2r)
            _phase3(nc, tc, AF, OP, t_, bias, x2t,
                    x2m if FP8_MLP else x2r)


def _phase12(nc, tc, AF, OP, t_, bias, gt, md, ones_r,
             keep, pw, p2w, psP, psS, psA, psB, x2t, x2r):
    """Q/K/V projections fused with attention; writes x2t/x2r."""
    # ---- persistent tiles
    xqt = [keep.tile([128, 2 * CH], F32, tag=f"xqt{f}", name=f"xqt{f}")
           for f in range(8)]
    xqtb = [keep.tile([128, 2 * CH], F16, tag=f"xqtb{f}", name=f"xqtb{f}")
            for f in range(8)]
    xbt = [keep.tile([128, T], F16, tag=f"xbt{f}", name=f"xbt{f}")
           for f in range(8)]
    qt = [keep.tile([128, 2 * CH], F16, tag=f"qt{f}", name=f"qt{f}")
          for f in range(8)]
    vaug = [keep.tile([128, H, D + 1], F16, tag=f"va{rt}", name=f"va{rt}")
            for rt in range(T // 128)]

    # ---- input DMAs (all contiguous rows)
    for f in range(8):
        nc.sync.dma_start(xqtb[f][:], t_["xqtb"][f * 128:(f + 1) * 128, :])
    # xbt on the gpsimd queue (parallel with sync), in column halves so the
    # first K/V projection groups can start before the full 4MB lands
    for cb in range(2):
        for f in range(8):
            nc.gpsimd.dma_start(
                xbt[f][:, cb * 1024:(cb + 1) * 1024],
                t_["xbt"][f * 128:(f + 1) * 128, cb * 1024:(cb + 1) * 1024])
    # xqt (residual operand) is not needed until the first finalize; keep it
    # off the sync queue so weight stages are not delayed behind it
    for f in range(8):
        nc.gpsimd.dma_start(xqt[f][:], t_["xqt"][f * 128:(f + 1) * 128, :])

    # vaug augmentation column: [v|1] for every head
    for rt in range(T // 128):
        nc.vector.memset(vaug[rt][:, :, D:D + 1].bitcast(U16), 0x3C00)

    # fold the V-projection bias into the residual operand (softmax
    # weights sum to 1, so attn(v + bv) = attn(v) + bv)
    for f in range(8):
        nc.vector.tensor_scalar_add(xqt[f][:], xqt[f][:],
                                    bias["bv"][:, f:f + 1])

    ev_pend = [None]

    def defer_ev(fn):
        if ev_pend[0] is not None:
            ev_pend[0]()
        ev_pend[0] = fn

    def drain_ev():
        if ev_pend[0] is not None:
            ev_pend[0]()
        ev_pend[0] = None

    # ---- Q projection: qt[f] = (Wq.T @ xq.T)*0.125 + bqs  (fp16 out)
    wqs = {}

    def load_wq(f):
        wqs[f] = pw.tile([128, 8, 128], F16, tag="wqk", bufs=4, name=f"wq{f}")
        nc.sync.dma_start(
            wqs[f][:].rearrange("p k n -> p (k n)"),
            t_["wq_t"][f, :, :, :].rearrange("p k n -> p (k n)"))

    load_wq(0)
    load_wq(1)
    for f in range(8):
        if f + 2 < 8:
            load_wq(f + 2)
        wq = wqs.pop(f)
        pp = psP.tile([128, 2 * CH], F32, tag="proj")
        for kc in range(8):
            nc.tensor.matmul(pp[:], wq[:, kc, :], xqtb[kc][:],
                             start=(kc == 0), stop=(kc == 7))
        defer_ev(lambda pp=pp, f=f: nc.scalar.activation(
            qt[f][:], pp[:], AF.Identity,
            bias=bias["bqs"][:, f:f + 1], scale=0.125))

    # ---- V projection for one 256-wide column unit (4 heads: 4u..4u+3).
    # Unit u is first consumed by attention step f = 2u, so later units can
    # interleave as background PE work deep into the attention phase.
    def emit_vproj_groups(u):
        """Returns a list of thunks; each emits one rt-group (8 matmuls)."""
        wvs = [pw.tile([128, 256], F16, tag="wv", bufs=16, name=f"wv{u}_{i}")
               for i in range(8)]
        for kc in range(8):
            nc.sync.dma_start(wvs[kc][:], t_["wv_t"][u, kc, :, :])

        def mk(rt):
            def go():
                pp = psP.tile([128, 512], F32, tag="proj")
                for kc in range(8):
                    nc.tensor.matmul(
                        pp[:, 0:256],
                        xbt[kc][:, rt * 128:(rt + 1) * 128], wvs[kc][:],
                        start=(kc == 0), stop=(kc == 7))

                def ev(pp=pp, rt=rt):
                    nc.vector.tensor_copy(
                        vaug[rt][:, 4 * u:4 * u + 4, 0:D],
                        pp[:, 0:256].rearrange("p (h d) -> p h d", d=D))
                defer_ev(ev)
            return go
        return [mk(rt) for rt in range(T // 128)]

    # ---- K projection for one feature block f (4 rb-groups of 8 matmuls)
    wks = {}

    def load_wk(f):
        wks[f] = pw.tile([128, 8, 128], F16, tag="wqk", bufs=4, name=f"wk{f}")
        nc.sync.dma_start(
            wks[f][:].rearrange("p k n -> p (k n)"),
            t_["wk_t"][f, :, :, :].rearrange("p k n -> p (k n)"))

    def emit_kproj_groups(f, kth_f):
        wk = wks.pop(f)

        def mk(rb):
            def go():
                pp = psP.tile([128, 512], F32, tag="proj")
                for kc in range(8):
                    nc.tensor.matmul(pp[:], wk[:, kc, :],
                                     xbt[kc][:, rb * 512:(rb + 1) * 512],
                                     start=(kc == 0), stop=(kc == 7))
                defer_ev(lambda pp=pp, rb=rb: nc.vector.tensor_scalar_add(
                    kth_f[:, rb * 512:(rb + 1) * 512], pp[:],
                    bias["bk"][:, f:f + 1]))
            return go
        return [mk(rb) for rb in range(4)]

    # ---- emit: V(u0), K(0) up-front; then per-f attention with K(f+1)
    # and V(u1..u3) groups interleaved at unit boundaries.
    for thunk in emit_vproj_groups(0):
        thunk()
    load_wk(0)
    kth = [None, None]
    kth[0] = keep.tile([128, T], F16, tag="kth", bufs=2, name="kth0")
    for thunk in emit_kproj_groups(0, kth[0]):
        thunk()
    drain_ev()

    bg = []                # background PE work: (tag, thunk)
    pending = []           # AV matmuls deferred ~one block
    fin_q = []             # finalize chains deferred one (hp,qi) unit

    def flush():
        while len(pending) > 2:
            pending.pop(0)()
        while len(fin_q) > 1:
            fin_q.pop(0)()

    def bg_due(tag, f):
        kind, idx = tag
        return (kind == "k" and idx <= f) or (kind == "v" and f >= 2 * idx)

    for f in range(8):
        # schedule next-step projection work into this step's attention
        if f + 1 < 8:
            load_wk(f + 1)
            kth[(f + 1) % 2] = keep.tile([128, T], F16, tag="kth", bufs=2,
                                         name=f"kth{f + 1}")
            bg.extend((("k", f + 1), t)
                      for t in emit_kproj_groups(f + 1, kth[(f + 1) % 2]))
        if f in (0, 2, 4):
            u = f // 2 + 1
            bg.extend((("v", u), t) for t in emit_vproj_groups(u))
        # anything attention(f) depends on MUST be emitted before it; pop
        # from the front until no due entries remain, then flush the last
        # deferred PSUM evacuation
        while any(bg_due(tag, f) for tag, _ in bg):
            bg.pop(0)[1]()
        drain_ev()
        kth_f = kth[f % 2]

        unit_i = 0
        for hp in range(2):
            h = 2 * f + hp
            po = 64 * hp
            # one accumulator bank holds both q-chunks: A in [:, 0, :],
            # B in [:, 1, :] — the whole finalize then runs once at width 512
            acc = psA.tile([128, 2, CH], F32, tag="acc")
            for (qi, qoff, nblk) in ((0, 0, 4), (1, CH, 8)):
                diag = nblk - 1
                for blk in range(nblk):
                    ps = psS.tile([128, 2, CH], F32, tag="ps")
                    for s in range(2):
                        c = 2 * blk + s
                        nc.tensor.matmul(
                            ps[:, s, :],
                            kth_f[po:po + D, c * 128:(c + 1) * 128],
                            qt[f][po:po + D, qoff:qoff + CH],
                            start=True, stop=True)
                    ex = p2w.tile([128, 2, CH], F16, tag="ex", bufs=6)
                    if blk == diag:
                        sm = p2w.tile([128, 2, CH], F32, tag="sm", bufs=3)
                        nc.vector.tensor_tensor(out=sm[:], in0=ps[:],
                                                in1=md[:], op=OP.add)
                        nc.scalar.activation(ex[:], sm[:], AF.Exp)
                    else:
                        nc.scalar.activation(
                            ex[:], ps[:], AF.Exp,
                            bias=gt[:, 2 * blk + qi:2 * blk + qi + 1])
                    flush()

                    def mk_avs(ex=ex, blk=blk, h=h, acc=acc, qi=qi,
                               nblk=nblk):
                        def go():
                            for s in range(2):
                                c = 2 * blk + s
                                nc.tensor.matmul(
                                    acc[0:D + 1, qi, :], vaug[c][:, h, :],
                                    ex[:, s, :],
                                    start=(c == 0), stop=(c == 2 * nblk - 1))
                        return go
                    pending.append(mk_avs())

                while pending:
                    pending.pop(0)()

                # interleave background projection work between units,
                # paced so everything due by f+1 is emitted by end of f
                units_left = 4 - unit_i
                n_due = sum(1 for tag, _ in bg if bg_due(tag, f + 1))
                npop = max(1 if bg else 0, -(-n_due // units_left))
                for _ in range(npop):
                    if bg:
                        bg.pop(0)[1]()
                unit_i += 1

            def mk_fin(acc=acc, f=f, hp=hp):
                def go():
                    lr = p2w.tile([128, 2, CH], F32, tag="lr", bufs=2)
                    nc.scalar.activation(lr[D:D + 1, :, :],
                                         acc[D:D + 1, :, :], AF.Ln)
                    rec = p2w.tile([128, 2, CH], F32R, tag="rec", bufs=2)
                    nc.scalar.activation(rec[D:D + 1, :, :],
                                         lr[D:D + 1, :, :], AF.Exp,
                                         scale=-1.0)
                    pb = psB.tile([128, 2, CH], F32, tag="pb")
                    nc.tensor.matmul(pb[0:D, :, :],
                                     ones_r[D:D + 1, :],
                                     rec[D:D + 1, :, :],
                                     start=True, stop=True)
                    pbs = p2w.tile([128, 2, CH], F32, tag="pbs", bufs=2)
                    nc.vector.tensor_copy(pbs[0:D, :, :], pb[0:D, :, :])
                    tt = p2w.tile([128, 2, CH], F32, tag="tt", bufs=2)
                    nc.vector.tensor_tensor(
                        out=tt[0:D, :, :], in0=acc[0:D, :, :],
                        in1=pbs[0:D, :, :], op=OP.mult)
                    if hp == 0:
                        nc.vector.tensor_tensor(
                            out=x2t[f][0:D, :],
                            in0=tt[0:D, :, :].rearrange("p a b -> p (a b)"),
                            in1=xqt[f][0:D, :], op=OP.add)
                    else:
                        # shift raw attn-out to partitions 64:128, then do
                        # the residual add partition-aligned up there
                        nc.gpsimd.dma_start(
                            x2t[f][D:128, :],
                            tt[0:D, :, :].rearrange("p a b -> p (a b)"))
                        nc.vector.tensor_tensor(
                            out=x2t[f][D:128, :],
                            in0=x2t[f][D:128, :],
                            in1=xqt[f][D:128, :], op=OP.add)
                return go
            fin_q.append(mk_fin())
        # end hp loop: cast x2t[f] -> fp16 once all 4 units finalized
        def mk_cast(f=f):
            def go():
                nc.vector.tensor_copy(x2r[f][:], x2t[f][:])
            return go
        fin_q.append(mk_cast())

    while bg:
        bg.pop(0)[1]()
    while pending:
        pending.pop(0)()
    for fn in fin_q:
        fn()
    fin_q.clear()
    drain_ev()


def _phase3(nc, tc, AF, OP, t_, bias, x2t, x2m):
    """MLP (transposed) + residual; output stays transposed (host fixes).

    x2m: fp8 DoubleRow-paired tiles [128, 2, 512] x4 when FP8_MLP, else
    plain fp16 tiles [128, 512] x8."""
    w1_t, w2_t, out_t = t_["w1_t"], t_["w2_t"], t_["out_t"]
    DR = mybir.MatmulPerfMode.DoubleRow
    osc = 1.0 / W8SCALE if FP8_MLP else 1.0
    with tc.tile_pool(name="p3h", bufs=1) as p3h, \
         tc.tile_pool(name="p3w", bufs=2) as p3w, \
         tc.tile_pool(name="p3y", bufs=2) as p3y, \
         tc.tile_pool(name="ps3", bufs=4, space="PSUM") as ps3:

        if FP8_MLP:
            # h, DoubleRow-paired: ht[kc2][:, i, :] = hidden block 2*kc2+i
            ht = [p3h.tile([128, 2, 2 * CH], F8, tag=f"ht{i}", name=f"ht{i}")
                  for i in range(16)]
        else:
            ht = [p3h.tile([128, 4, 2 * CH], F16, tag=f"ht{i}", name=f"ht{i}")
                  for i in range(8)]
        w1ss = {}

        def load_w1(m):
            if FP8_MLP:
                w1ss[m] = p3w.tile([128, 4, 2, 128], F8, tag="w1s",
                                   bufs=3, name=f"w1s{m}")
                nc.sync.dma_start(
                    w1ss[m][:].rearrange("p a b c -> p (a b c)"),
                    w1_t[m].rearrange("p a b c -> p (a b c)"))
            else:
                w1ss[m] = p3w.tile([128, 8, 128], F16, tag="w1s",
                                   bufs=3, name=f"w1s{m}")
                nc.sync.dma_start(
                    w1ss[m][:].rearrange("p k n -> p (k n)"),
                    w1_t[m].rearrange("p k n -> p (k n)"))

        load_w1(0)
        load_w1(1)
        ev_pend = [None]
        for m in range(32):
            if m + 2 < 32:
                load_w1(m + 2)
            w1s = w1ss.pop(m)
            pp = ps3.tile([128, 2 * CH], F32, tag="proj")
            if FP8_MLP:
                for kc2 in range(4):
                    nc.tensor.matmul(pp[:], w1s[:, kc2, :, :], x2m[kc2][:],
                                     start=(kc2 == 0), stop=(kc2 == 3),
                                     perf_mode=DR)
            else:
                for kc in range(8):
                    nc.tensor.matmul(pp[:], w1s[:, kc, :], x2m[kc][:],
                                     start=(kc == 0), stop=(kc == 7))
            if ev_pend[0] is not None:
                ev_pend[0]()
            hslot = ht[m // 2][:, m % 2, :] if FP8_MLP \
                else ht[m // 4][:, m % 4, :]
            ev_pend[0] = (lambda pp=pp, m=m, hslot=hslot: nc.scalar.activation(
                hslot, pp[:], AF.Relu,
                bias=bias["b1"][:, m:m + 1], scale=osc))
        ev_pend[0]()
        ev_pend[0] = None

        w2ss = {}

        def load_w2(mo):
            if FP8_MLP:
                w2ss[mo] = p3w.tile([128, 16, 2, 128], F8, tag="w2s",
                                    bufs=2, name=f"w2s{mo}")
                nc.sync.dma_start(
                    w2ss[mo][:].rearrange("p a b c -> p (a b c)"),
                    w2_t[mo].rearrange("p a b c -> p (a b c)"))
            else:
                w2ss[mo] = p3w.tile([128, 32, 128], F16, tag="w2s",
                                    bufs=2, name=f"w2s{mo}")
                nc.sync.dma_start(
                    w2ss[mo][:].rearrange("p k n -> p (k n)"),
                    w2_t[mo].rearrange("p k n -> p (k n)"))

        load_w2(0)
        load_w2(1)
        for mo in range(8):
            if mo + 2 < 8:
                load_w2(mo + 2)
            w2s = w2ss.pop(mo)
            pp = ps3.tile([128, 2 * CH], F32, tag="proj")
            if FP8_MLP:
                for kc2 in range(16):
                    nc.tensor.matmul(pp[:], w2s[:, kc2, :, :], ht[kc2][:],
                                     start=(kc2 == 0), stop=(kc2 == 15),
                                     perf_mode=DR)
            else:
                for kc in range(32):
                    nc.tensor.matmul(pp[:], w2s[:, kc, :],
                                     ht[kc // 4][:, kc % 4, :],
                                     start=(kc == 0), stop=(kc == 31))
            ys = p3y.tile([128, 2 * CH], F32, tag="ys", bufs=2, name=f"ys{mo}")
            nc.scalar.activation(ys[:], pp[:], AF.Identity,
                                 bias=bias["b2"][:, mo:mo + 1], scale=osc)
            nc.vector.tensor_tensor(out=ys[:], in0=ys[:], in1=x2t[mo][:],
                                    op=OP.add)
            nc.gpsimd.dma_start(out_t[mo * 128:(mo + 1) * 128, :], ys[:])


# --------------------------------------------------------------- host driver
def _install_ntff_hook():
    """The container's antenv stub lacks axon_hooks; provide it so
    run_bass_kernel_spmd(trace=True) can capture NTFF profiles via libaxon."""
    import types

    try:
        import antenv.axon_hooks  # noqa: F401
        return
    except ImportError:
        pass
    holder = {"h": None}
    mod = types.ModuleType("antenv.axon_hooks")
    mod.set_axon_ntff_profile_hook = lambda h: holder.__setitem__("h", h)
    mod.get_axon_ntff_profile_hook = lambda: holder["h"]
    sys.modules["antenv.axon_hooks"] = mod
    import antenv

    antenv.axon_hooks = mod
    if "/root/.axon_site" not in sys.path:
        sys.path.insert(0, "/root/.axon_site")
    from trn_agent_boot.trn_boot import _ntff_profile_via_ctypes

    so = "/opt/axon/libaxon_pjrt.so"
    if os.path.exists(so):
        mod.set_axon_ntff_profile_hook(_ntff_profile_via_ctypes(so))


def _get_program():
    key = ("v6", FP8_MLP)
    if key not in _prog_cache:
        nc = _build_program()
        _legalize_waits(nc)
        _prog_cache[key] = nc
    return _prog_cache[key]


def _prep_shared(Wqkv, W1, W2, bqkv, b1, b2):
    Wq, Wk, Wv = Wqkv[:, :N], Wqkv[:, N:2 * N], Wqkv[:, 2 * N:]
    wq_t = np.ascontiguousarray(
        Wq.reshape(8, 128, 8, 128).transpose(2, 1, 0, 3).astype(np.float16))
    wk_t = np.ascontiguousarray(
        Wk.reshape(8, 128, 8, 128).transpose(2, 1, 0, 3).astype(np.float16))
    wv_t = np.ascontiguousarray(
        Wv.reshape(8, 128, 4, 256).transpose(2, 0, 1, 3).astype(np.float16))
    if FP8_MLP:
        import ml_dtypes
        f8 = ml_dtypes.float8_e4m3
        w1_t = np.ascontiguousarray(
            (W1 * W8SCALE).reshape(4, 2, 128, 32, 128)
            .transpose(3, 2, 0, 1, 4).astype(f8))
        w2_t = np.ascontiguousarray(
            (W2 * W8SCALE).reshape(16, 2, 128, 8, 128)
            .transpose(3, 2, 0, 1, 4).astype(f8))
    else:
        w1_t = np.ascontiguousarray(
            W1.reshape(8, 128, 32, 128).transpose(2, 1, 0, 3)
            .astype(np.float16))
        w2_t = np.ascontiguousarray(
            W2.reshape(32, 128, 8, 128).transpose(2, 1, 0, 3)
            .astype(np.float16))
    def pf(v):
        # bias layout [128, w]: element [p, f] = v[f*128 + p]
        return np.ascontiguousarray(v.reshape(-1, 128).T)

    return {
        "wq_t": wq_t, "wk_t": wk_t, "wv_t": wv_t,
        "w1_t": w1_t, "w2_t": w2_t,
        "bqs": pf(bqkv[:N] * 0.125),
        "bk": pf(bqkv[N:2 * N]),
        "bv": pf(bqkv[2 * N:]),
        "b1": pf(b1), "b2": pf(b2),
    }


def _core_chunks(c):
    b, j = c // 4, c % 4
    return b, j, 7 - j


def _slot_blocks(j):
    # slot order of the 8 kv row-blocks: slot 3 = A diag (block j),
    # slot 7 = B diag (block 7-j), others ascending.
    other = [b for b in range(8) if b not in (j, 7 - j)]
    return [other[0], other[1], other[2], j, other[3], other[4], other[5],
            7 - j]


def _make_gates(j):
    slots = _slot_blocks(j)
    g = np.full((128, 16), -1e9, np.float32)
    for s in range(8):
        if s != 3 and slots[s] < j:
            g[:, 2 * s] = 0.0          # allowed for A
        if s != 7 and slots[s] < 7 - j:
            g[:, 2 * s + 1] = 0.0      # allowed for B
    return g


_MASKD = np.where(np.arange(256)[:, None] <= np.arange(CH)[None, :],
                  0.0, -1e9).astype(np.float32)


def kernel(x, Wqkv, bqkv, W1, b1, W2, b2, _trace=False):
    x = np.asarray(x, dtype=np.float32)
    shared = _prep_shared(np.asarray(Wqkv, np.float32),
                          np.asarray(W1, np.float32),
                          np.asarray(W2, np.float32),
                          np.asarray(bqkv, np.float32),
                          np.asarray(b1, np.float32),
                          np.asarray(b2, np.float32))
    in_maps = []
    for c in range(NCORES):
        b, j, jb = _core_chunks(c)
        xqc = np.concatenate(
            [x[b, j * CH:(j + 1) * CH], x[b, jb * CH:(jb + 1) * CH]], axis=0)
        xqt = np.ascontiguousarray(xqc.T)
        xbp = x[b].reshape(8, CH, N)[_slot_blocks(j)].reshape(T, N)
        in_maps.append({
            **shared,
            "xqt": xqt,
            "xqtb": np.ascontiguousarray(xqt.astype(np.float16)),
            "xbt": np.ascontiguousarray(xbp.T.astype(np.float16)),
            "gates": _make_gates(j), "maskd": _MASKD,
        })

    nc = _get_program()
    if _trace:
        _install_ntff_hook()
    res = run_bass_kernel_spmd(nc, in_maps, list(range(NCORES)), trace=_trace)

    outf = np.empty((B, T, N), dtype=np.float32)
    for c in range(NCORES):
        b, j, jb = _core_chunks(c)
        o = np.ascontiguousarray(res.results[c]["out_t"].T)
        outf[b, j * CH:(j + 1) * CH] = o[:CH]
        outf[b, jb * CH:(jb + 1) * CH] = o[CH:]
    if _trace:
        kernel.last_results = res
    return outf
